# revision 14
# baseline (speedup 1.0000x reference)
"""Bass/Tile TRN2 kernel for BitLinear causal self-attention (B=4, T=1024, C=1024, H=16).

Sharding: tensor-parallel over heads (2 heads/core, 8 cores) for qkv+attention.
y is resharded to row (token) shards for the output projection via two
AllToAlls split by head-half (the first overlaps attention) plus one tiny
AllGather for the second BitLinear's layernorm stats (hi/lo bf16 split).
qkv projection, QK+exp, and PV are software-pipelined per batch sample.
"""

import functools
import math
from contextlib import ExitStack

import ml_dtypes
import numpy as np

import concourse.bacc as bacc
import concourse.bass as bass
import concourse.mybir as mybir
import concourse.tile as tile
from concourse import masks as masks_mod
from concourse.bass_utils import run_bass_kernel_spmd

B, T, C = 4, 1024, 1024
H, HD = 16, 64
NCORES = 8
HPC = H // NCORES
TOK = B * T
RPC = TOK // NCORES
QB = 128.0
EPS = 1e-5

BF16 = mybir.dt.bfloat16
F32 = mybir.dt.float32
AF = mybir.ActivationFunctionType
ALU = mybir.AluOpType
AX = mybir.AxisListType


def _emit(nc, tc, ctx):
    qxT = nc.dram_tensor("qxT", [C, TOK], BF16, kind="ExternalInput")
    qwinT = nc.dram_tensor("qwinT", [C, 3 * HPC * HD], BF16, kind="ExternalInput")
    qwoutT = nc.dram_tensor("qwoutT", [C, C], BF16, kind="ExternalInput")
    consts = nc.dram_tensor("consts", [1, 8], F32, kind="ExternalInput")
    bsel = nc.dram_tensor("bsel", [1, 8], F32, kind="ExternalInput")
    msel = nc.dram_tensor("msel", [1, 8], F32, kind="ExternalInput")
    out = nc.dram_tensor("out", [RPC, C], F32, kind="ExternalOutput")

    singles = ctx.enter_context(tc.tile_pool(name="singles", bufs=1))
    big = ctx.enter_context(tc.tile_pool(name="big", bufs=3, space="PSUM"))
    small = ctx.enter_context(tc.tile_pool(name="small", bufs=2, space="PSUM"))
    sb = ctx.enter_context(tc.tile_pool(name="sb", bufs=2))
    dram = ctx.enter_context(tc.tile_pool(name="dram", bufs=1, space="DRAM"))

    # ---- setup ----
    ident_bf = singles.tile([128, 128], BF16)
    masks_mod.make_identity(nc, ident_bf[:])
    ident_f32 = singles.tile([128, 128], F32)
    masks_mod.make_identity(nc, ident_f32[:])

    ones_row = singles.tile([1, 128], F32)
    nc.vector.memset(ones_row[:], 1.0)
    ones_col = singles.tile([128, 1], F32)
    nc.vector.memset(ones_col[:], 1.0)
    ones16 = singles.tile([16, 1], F32)
    nc.vector.memset(ones16[:], 1.0)

    csb = singles.tile([1, 8], F32)
    nc.sync.dma_start(csb[:], consts[:])
    bsel_sb = singles.tile([1, 8], F32)
    nc.sync.dma_start(bsel_sb[:], bsel[:])
    msel_sb = singles.tile([1, 8], F32)
    nc.sync.dma_start(msel_sb[:], msel[:])

    cb_ps = small.tile([128, 8], F32, tag="small")
    nc.tensor.matmul(cb_ps[:], ones_row[:], csb[:])
    cbc = singles.tile([128, 8], F32)
    nc.vector.tensor_copy(cbc[:], cb_ps[:])

    qwin_all = singles.tile([128, 8 * 384], BF16)
    nc.sync.dma_start(qwin_all[:], qwinT.rearrange("(c p) o -> p c o", p=128))

    def qwin(c, lo, hi):
        return qwin_all[:, c * 384 + lo:c * 384 + hi]

    qT_sb = singles.tile([128, TOK], BF16)
    kT_sb = singles.tile([128, TOK], BF16)
    vT_sb = singles.tile([128, TOK], BF16)

    qxT_r = qxT.rearrange("(c p) t -> p c t", p=128)

    # collective buffers
    a2a1_in = dram.tile([NCORES * 64, 512], BF16)
    a2a1_out = dram.tile([NCORES * 64, 512], BF16)
    a2a2_in = dram.tile([NCORES * 64, 512], BF16)
    a2a2_out = dram.tile([NCORES * 64, 512], BF16)
    ag_in = dram.tile([2, 512], BF16)
    ag_out = dram.tile([16, 512], BF16)
    a2a1_in_r = a2a1_in.rearrange("(bb h p) t -> p bb h t", p=64, h=2)
    a2a2_in_r = a2a2_in.rearrange("(bb h p) t -> p bb h t", p=64, h=2)

    yT_sb = singles.tile([128, TOK], BF16)
    stats = singles.tile([128, 9], F32)
    sq_tmp = singles.tile([128, 512], BF16)

    va = []
    for tb32 in range(32):
        t_ = singles.tile([128, 2 * (HD + 1)], BF16, tag=f"va{tb32}", name=f"va{tb32}")
        nc.vector.memset(t_[:, HD:HD + 1], 1.0)
        nc.vector.memset(t_[:, 2 * HD + 1:2 * HD + 2], 1.0)
        va.append(t_)

    def emit_qkv(b):
        for tb in (2 * b, 2 * b + 1):
            if tb == 0:
                qx_tb = sb.tile([128, 8, 512], BF16, tag="qx", bufs=3, name=f"qx{tb}")
                for c in range(8):
                    nc.sync.dma_start(qx_tb[:, c, :], qxT_r[:, c, 0:512])
            else:
                qx_tb = sb.tile([128, 8, 512], BF16, tag="qx", bufs=3, name=f"qx{tb}")
                nc.sync.dma_start(qx_tb[:], qxT_r[:, :, tb * 512:(tb + 1) * 512])
            qk_ps = big.tile([128, 1024], F32, tag="big", name=f"qkps{tb}")
            v_ps = big.tile([128, 512], F32, tag="big", name=f"vps{tb}")
            for c in range(8):
                st, sp = (c == 0), (c == 7)
                nc.tensor.matmul(qk_ps[:, 0:512], qwin(c, 0, 128), qx_tb[:, c, :], start=st, stop=sp)
                nc.tensor.matmul(qk_ps[:, 512:1024], qwin(c, 128, 256), qx_tb[:, c, :], start=st, stop=sp)
                nc.tensor.matmul(v_ps[:], qwin(c, 256, 384), qx_tb[:, c, :], start=st, stop=sp)
            nc.vector.tensor_copy(qT_sb[:, tb * 512:(tb + 1) * 512], qk_ps[:, 0:512])
            nc.vector.tensor_copy(kT_sb[:, tb * 512:(tb + 1) * 512], qk_ps[:, 512:1024])
            nc.vector.tensor_copy(vT_sb[:, tb * 512:(tb + 1) * 512], v_ps[:])
        for tb32 in range(8 * b, 8 * b + 8):
            tr_ps = small.tile([128, 128], BF16, tag="small", name=f"vtr{tb32}")
            nc.tensor.transpose(tr_ps[:], vT_sb[:, tb32 * 128:(tb32 + 1) * 128], ident_bf[:])
            nc.vector.tensor_copy(va[tb32][:, 0:HD], tr_ps[:, 0:HD])
            nc.vector.tensor_copy(va[tb32][:, HD + 1:2 * HD + 1], tr_ps[:, HD:2 * HD])

    def emit_qk(hl, b, pair_idx):
        """QK^T, exp, causal select. Returns se tile dict."""
        qrow = hl * HD
        tbase = b * T
        se_tiles = {}
        for ib in range(2):
            jb_max = 4 * ib + 3
            for jp in range(0, (jb_max + 1) // 2):
                jb0, jb1 = 2 * jp, 2 * jp + 1
                s_ps = big.tile([128, 1024], F32, tag="big", name=f"s_ps{pair_idx}_{ib}_{jp}")
                for col, jb in ((0, jb0), (512, jb1)):
                    nc.tensor.matmul(
                        s_ps[:, col:col + 512],
                        kT_sb[qrow:qrow + HD, tbase + jb * 128:tbase + (jb + 1) * 128],
                        qT_sb[qrow:qrow + HD, tbase + ib * 512:tbase + (ib + 1) * 512],
                    )
                se = sb.tile([128, 1024], BF16, tag="se", bufs=14, name=f"se{pair_idx}_{ib}_{jp}")
                nc.scalar.activation(se[:], s_ps[:], AF.Exp, scale=cbc[:, 0:1])
                for col, jb in ((0, jb0), (512, jb1)):
                    p = jb - 4 * ib
                    if 0 <= p <= 3:
                        dcol = col + 128 * p
                        nc.gpsimd.affine_select(
                            out=se[:, dcol:dcol + 128], in_=se[:, dcol:dcol + 128],
                            compare_op=ALU.is_ge, fill=0.0, base=0,
                            pattern=[[1, 128]], channel_multiplier=-1,
                        )
                se_tiles[(ib, jp)] = se
        return se_tiles

    def emit_pv(hl, b, pair_idx, se_tiles):
        """PV, softmax divide, y transposes, A2A staging, stats."""
        tbase = b * T
        y_pair = sb.tile([128, 512], BF16, tag="ypair", bufs=4, name=f"ypair{pair_idx}")
        for ib128 in range(8):
            ib512 = ib128 // 4
            icol = 128 * (ib128 % 4)
            pv_ps = small.tile([128, HD + 1], F32, tag="small", name=f"pv{pair_idx}_{ib128}")
            for jb in range(ib128 + 1):
                se = se_tiles[(ib512, jb // 2)]
                lhs = se[:, 512 * (jb % 2) + icol: 512 * (jb % 2) + icol + 128]
                nc.tensor.matmul(
                    pv_ps[:], lhs, va[b * 8 + jb][:, (HD + 1) * hl:(HD + 1) * hl + HD + 1],
                    start=(jb == 0), stop=(jb == ib128),
                )
            rec = sb.tile([128, 1], F32, tag="rec", bufs=2, name=f"rec{pair_idx}_{ib128}")
            nc.vector.reciprocal(rec[:], pv_ps[:, HD:HD + 1])
            nc.vector.tensor_scalar_mul(y_pair[:, ib128 * HD:(ib128 + 1) * HD], pv_ps[:, 0:HD], rec[:])
        for ib128 in range(8):
            ytr_ps = small.tile([HD, 128], BF16, tag="small", name=f"ytr{pair_idx}_{ib128}")
            nc.tensor.transpose(ytr_ps[:], y_pair[:, ib128 * HD:(ib128 + 1) * HD], ident_bf[:])
            nc.vector.tensor_copy(
                yT_sb[hl * HD:(hl + 1) * HD, tbase + ib128 * 128:tbase + (ib128 + 1) * 128],
                ytr_ps[:],
            )
        if hl == 0:
            nc.sync.dma_start(a2a1_in_r[:, b, :, :], yT_sb[0:64, tbase:tbase + T])
        else:
            nc.sync.dma_start(a2a2_in_r[:, b, :, :], yT_sb[64:128, tbase:tbase + T])
        s1 = sb.tile([128, 1], F32, tag="st1", bufs=2, name=f"s1_{pair_idx}")
        nc.vector.reduce_sum(s1[:], y_pair[:], axis=AX.X)
        nc.vector.tensor_mul(sq_tmp[:], y_pair[:], y_pair[:])
        s2 = sb.tile([128, 1], F32, tag="st2", bufs=2, name=f"s2_{pair_idx}")
        nc.vector.reduce_sum(s2[:], sq_tmp[:], axis=AX.X)
        s3 = sb.tile([128, 1], F32, tag="st3", bufs=2, name=f"s3_{pair_idx}")
        nc.vector.reduce_max(s3[:], y_pair[:], axis=AX.X, apply_absolute_value=True)
        if hl == 0:
            nc.vector.tensor_copy(stats[:, b:b + 1], s1[:])
            nc.vector.tensor_copy(stats[:, 4 + b:5 + b], s2[:])
        else:
            nc.vector.tensor_add(stats[:, b:b + 1], stats[:, b:b + 1], s1[:])
            nc.vector.tensor_add(stats[:, 4 + b:5 + b], stats[:, 4 + b:5 + b], s2[:])
        if pair_idx == 0:
            nc.vector.tensor_copy(stats[:, 8:9], s3[:])
        else:
            nc.vector.tensor_max(stats[:, 8:9], stats[:, 8:9], s3[:])

    # ---- software-pipelined schedule: PV lags QK by one pair ----
    # pair order: (0,0) (1,0) (0,1) (0,2) (0,3) | A2A1 | (1,1) (1,2) (1,3)
    order = [(0, 0), (1, 0), (0, 1), (0, 2), (0, 3), (1, 1), (1, 2), (1, 3)]
    qkv_before = {0: [0], 2: [1], 3: [2], 4: [3]}  # emit_qkv(b) before pair slot i
    pending = None  # (hl, b, pair_idx, se_tiles)
    a2a1_done = False
    for i, (hl, b) in enumerate(order):
        for bb in qkv_before.get(i, []):
            emit_qkv(bb)
        se = emit_qk(hl, b, i)
        if pending is not None:
            emit_pv(*pending)
        pending = (hl, b, i, se)
        if i == 4:
            # after QK(0,3) emitted; PV(0,3) still pending -> flush it now so
            # the first A2A can launch
            emit_pv(*pending)
            pending = None
            nc.gpsimd.collective_compute(
                "AllToAll", ALU.bypass, replica_groups=[list(range(NCORES))],
                ins=[a2a1_in.opt()], outs=[a2a1_out.opt()],
            )
            qwout_all = singles.tile([128, 8 * 1024], BF16)
            nc.sync.dma_start(qwout_all[:], qwoutT.rearrange("(c p) o -> p c o", p=128))
            qy = singles.tile([128, 8, 512], BF16)
            a2a1_o_r = a2a1_out.rearrange("(j p) t -> p j t", p=64)
            a2a2_o_r = a2a2_out.rearrange("(j p) t -> p j t", p=64)
            nc.sync.dma_start(qy[0:64, :, :], a2a1_o_r[:, :, :])
    emit_pv(*pending)

    def qwout(c, lo, hi):
        return qwout_all[:, c * 1024 + lo:c * 1024 + hi]

    # ---- stats -> tiny AllGather (fires before the second A2A) ----
    st_ps = small.tile([1, 9], F32, tag="small")
    nc.tensor.matmul(st_ps[:], ones_col[:], stats[:])
    trm_ps = small.tile([1, 128], F32, tag="small")
    nc.tensor.transpose(trm_ps[:], stats[:, 8:9], ident_f32[:])
    gmax_l = singles.tile([1, 1], F32)
    nc.vector.reduce_max(gmax_l[:], trm_ps[:], axis=AX.X)

    srow = singles.tile([1, 512], F32)
    nc.vector.memset(srow[:], 0.0)
    nc.vector.tensor_copy(srow[:, 0:8], st_ps[:, 0:8])
    nc.vector.tensor_scalar_mul(srow[:, 8:16], msel_sb[:], gmax_l[:])
    srep_ps = small.tile([2, 512], F32, tag="small")
    nc.tensor.matmul(srep_ps[:], ones_row[:, 0:2], srow[:])
    srep = singles.tile([2, 512], F32)
    nc.vector.tensor_copy(srep[:], srep_ps[:])
    hi2 = singles.tile([2, 512], BF16)
    nc.vector.tensor_copy(hi2[:], srep[:])
    hi2f = singles.tile([2, 512], F32)
    nc.vector.tensor_copy(hi2f[:], hi2[:])
    lo2 = singles.tile([2, 512], BF16)
    nc.vector.tensor_sub(lo2[:], srep[:], hi2f[:])
    stag = singles.tile([2, 512], BF16)
    nc.vector.tensor_copy(stag[0:1, :], hi2[0:1, :])
    nc.vector.tensor_copy(stag[1:2, :], lo2[1:2, :])
    nc.sync.dma_start(ag_in[:], stag[:])
    nc.gpsimd.collective_compute(
        "AllGather", ALU.bypass, replica_groups=[list(range(NCORES))],
        ins=[ag_in.opt()], outs=[ag_out.opt()],
    )
    nc.gpsimd.collective_compute(
        "AllToAll", ALU.bypass, replica_groups=[list(range(NCORES))],
        ins=[a2a2_in.opt()], outs=[a2a2_out.opt()],
    )

    # ---- global stats scalar chain (overlaps the second A2A) ----
    ags = singles.tile([16, 16], BF16)
    nc.sync.dma_start(ags[:], ag_out[:, 0:16])
    agf = singles.tile([16, 16], F32)
    nc.vector.tensor_copy(agf[:], ags[:])
    agr = agf.rearrange("(j r) s -> j r s", r=2)
    stats_f = singles.tile([8, 16], F32)
    nc.vector.tensor_add(stats_f[:], agr[:, 0, :], agr[:, 1, :])

    glob_ps = small.tile([1, 16], F32, tag="small")
    nc.tensor.matmul(glob_ps[:], ones16[0:8, :], stats_f[:])
    sc = singles.tile([1, 24], F32)
    inv_tc = 1.0 / float(T * C)
    nc.vector.tensor_scalar_mul(sc[:, 0:8], glob_ps[:, 0:8], inv_tc)
    gmax = singles.tile([1, 1], F32)
    nc.vector.reduce_max(gmax[:], glob_ps[:, 8:16], axis=AX.X)
    nc.vector.tensor_mul(sc[:, 8:12], sc[:, 0:4], sc[:, 0:4])
    nc.vector.tensor_sub(sc[:, 8:12], sc[:, 4:8], sc[:, 8:12])
    nc.vector.tensor_scalar_add(sc[:, 8:12], sc[:, 8:12], 1e-5)
    sig = singles.tile([1, 4], F32)
    nc.scalar.activation(sig[:], sc[:, 8:12], AF.Sqrt)
    rsig = singles.tile([1, 4], F32)
    nc.vector.reciprocal(rsig[:], sig[:])
    nc.vector.tensor_scalar_mul(sc[:, 12:16], rsig[:], csb[:, 1:2])
    nc.vector.tensor_mul(sc[:, 16:20], sc[:, 0:4], sc[:, 12:16])
    tsel = singles.tile([1, 8], F32)
    nc.vector.tensor_mul(tsel[:], sc[:, 12:20], bsel_sb[:])
    row4 = singles.tile([1, 4], F32)
    nc.vector.reduce_sum(row4[:, 0:2], tsel.rearrange("p (g f) -> p g f", g=2), axis=AX.X)
    nc.vector.tensor_scalar_mul(row4[:, 3:4], gmax[:], csb[:, 2:3])
    nc.vector.tensor_scalar_mul(row4[:, 2:3], row4[:, 3:4], -1.0)
    qsc_ps = small.tile([128, 4], F32, tag="small")
    nc.tensor.matmul(qsc_ps[:], ones_row[:], row4[:])
    qsc = singles.tile([128, 4], F32)
    nc.vector.tensor_copy(qsc[:], qsc_ps[:])

    # quantize upper half (A2A1 data) while the second A2A is in flight
    qy_up = qy[0:64, :, :].rearrange("p j t -> p (j t)")
    nc.vector.tensor_scalar(out=qy_up, in0=qy_up, scalar1=qsc[0:64, 0:1], scalar2=qsc[0:64, 1:2],
                            op0=ALU.mult, op1=ALU.subtract)
    nc.vector.tensor_scalar(out=qy_up, in0=qy_up, scalar1=qsc[0:64, 2:3], scalar2=qsc[0:64, 3:4],
                            op0=ALU.max, op1=ALU.min)

    # lower half arrives with the second A2A
    nc.sync.dma_start(qy[64:128, :, :], a2a2_o_r[:, :, :])
    qy_lo = qy[64:128, :, :].rearrange("p j t -> p (j t)")
    nc.vector.tensor_scalar(out=qy_lo, in0=qy_lo, scalar1=qsc[64:128, 0:1], scalar2=qsc[64:128, 1:2],
                            op0=ALU.mult, op1=ALU.subtract)
    nc.vector.tensor_scalar(out=qy_lo, in0=qy_lo, scalar1=qsc[64:128, 2:3], scalar2=qsc[64:128, 3:4],
                            op0=ALU.max, op1=ALU.min)

    for tch in range(4):
        osb = sb.tile([128, 1024], F32, tag="ob", bufs=2, name=f"osb{tch}")
        for oh in range(2):
            o_ps = big.tile([128, 512], F32, tag="big", name=f"ops{tch}_{oh}")
            for cj in range(8):
                nc.tensor.matmul(
                    o_ps[:], qy[:, cj, tch * 128:(tch + 1) * 128],
                    qwout(cj, oh * 512, (oh + 1) * 512),
                    start=(cj == 0), stop=(cj == 7),
                )
            nc.vector.tensor_copy(osb[:, oh * 512:(oh + 1) * 512], o_ps[:])
        nc.sync.dma_start(out[tch * 128:(tch + 1) * 128, :], osb[:])


@functools.lru_cache(maxsize=1)
def build():
    nc = bacc.Bacc(None)
    with tile.TileContext(nc) as tc:
        with ExitStack() as ctx:
            _emit(nc, tc, ctx)
    nc.finalize()
    return nc


def _host_prep(x, w_in, w_out):
    x = np.asarray(x, np.float32)
    w_in = np.asarray(w_in, np.float32)
    w_out = np.asarray(w_out, np.float32)

    a1 = w_in.mean()
    qw1 = np.sign(w_in - a1).astype(np.float32)
    b1 = np.abs(w_in).mean()
    a2 = w_out.mean()
    qw2 = np.sign(w_out - a2).astype(np.float32)
    b2 = np.abs(w_out).mean()

    mu = x.mean(axis=(1, 2), keepdims=True)
    var = x.var(axis=(1, 2), keepdims=True)
    g1 = np.abs(x).max()
    xn = (x - mu) / np.sqrt(var + 1e-5)
    qx = np.clip(xn * (QB / g1), -QB + EPS, QB - EPS)
    scale1 = b1 * g1 / QB

    bf = ml_dtypes.bfloat16
    qxT = np.ascontiguousarray(qx.reshape(TOK, C).T).astype(bf)
    qwoutT = np.ascontiguousarray(qw2.T).astype(bf)
    att_scale = scale1 * scale1 / math.sqrt(HD)
    cbound = (QB - EPS) / QB * b2 * scale1
    consts = np.array([[att_scale, b2, cbound, 0, 0, 0, 0, 0]], np.float32)

    in_maps = []
    for core in range(NCORES):
        r0 = core * 128
        qwin = np.concatenate(
            [qw1[r0:r0 + 128], qw1[C + r0:C + r0 + 128], qw1[2 * C + r0:2 * C + r0 + 128]], axis=0
        )
        qwinT = np.ascontiguousarray(qwin.T).astype(bf)
        bsel_ = np.zeros((1, 8), np.float32)
        bsel_[0, core // 2] = 1.0
        bsel_[0, 4 + core // 2] = 1.0
        msel_ = np.zeros((1, 8), np.float32)
        msel_[0, core] = 1.0
        in_maps.append({
            "qxT": qxT, "qwinT": qwinT, "qwoutT": qwoutT,
            "consts": consts, "bsel": bsel_, "msel": msel_,
        })
    return in_maps


def kernel(x, w_in, w_out):
    in_maps = _host_prep(x, w_in, w_out)
    nc = build()
    res = run_bass_kernel_spmd(nc, in_maps, core_ids=list(range(NCORES)))
    out = np.concatenate([np.asarray(res.results[i]["out"]) for i in range(NCORES)], axis=0)
    return out.reshape(B, T, C).astype(np.float32)


# revision 15
# speedup vs baseline: 1.1772x; 1.1772x over previous
"""Bass/Tile TRN2 kernel for BitLinear causal self-attention (B=4, T=1024, C=1024, H=16).

Sharding: tensor-parallel over heads (2 heads/core, 8 cores) for qkv+attention.
y is resharded to row (token) shards for the output projection via two
AllToAlls split by head-half: all head-local-0 pairs run first so their
AllToAll fully overlaps the head-local-1 attention; the second AllToAll also
carries the per-core layernorm stats partials (hi/lo bf16 split) for the
second BitLinear. qkv, QK+exp+mask, and PV are software-pipelined.
"""

import functools
import math
from contextlib import ExitStack

import ml_dtypes
import numpy as np

import concourse.bacc as bacc
import concourse.bass as bass
import concourse.mybir as mybir
import concourse.tile as tile
from concourse import masks as masks_mod
from concourse.bass_utils import run_bass_kernel_spmd

B, T, C = 4, 1024, 1024
H, HD = 16, 64
NCORES = 8
HPC = H // NCORES
TOK = B * T
RPC = TOK // NCORES
QB = 128.0
EPS = 1e-5

BF16 = mybir.dt.bfloat16
F32 = mybir.dt.float32
AF = mybir.ActivationFunctionType
ALU = mybir.AluOpType
AX = mybir.AxisListType


def _emit(nc, tc, ctx):
    qxT = nc.dram_tensor("qxT", [C, TOK], BF16, kind="ExternalInput")
    qwinT = nc.dram_tensor("qwinT", [C, 3 * HPC * HD], BF16, kind="ExternalInput")
    qwoutT = nc.dram_tensor("qwoutT", [C, C], BF16, kind="ExternalInput")
    consts = nc.dram_tensor("consts", [1, 8], F32, kind="ExternalInput")
    bsel = nc.dram_tensor("bsel", [1, 8], F32, kind="ExternalInput")
    msel = nc.dram_tensor("msel", [1, 8], F32, kind="ExternalInput")
    out = nc.dram_tensor("out", [RPC, C], F32, kind="ExternalOutput")

    singles = ctx.enter_context(tc.tile_pool(name="singles", bufs=1))
    big = ctx.enter_context(tc.tile_pool(name="big", bufs=3, space="PSUM"))
    small = ctx.enter_context(tc.tile_pool(name="small", bufs=2, space="PSUM"))
    sb = ctx.enter_context(tc.tile_pool(name="sb", bufs=2))
    dram = ctx.enter_context(tc.tile_pool(name="dram", bufs=1, space="DRAM"))

    # ---- setup ----
    ident_bf = singles.tile([128, 128], BF16)
    masks_mod.make_identity(nc, ident_bf[:])
    ident_f32 = singles.tile([128, 128], F32)
    masks_mod.make_identity(nc, ident_f32[:])

    ones_row = singles.tile([1, 128], F32)
    nc.vector.memset(ones_row[:], 1.0)
    ones_col = singles.tile([128, 1], F32)
    nc.vector.memset(ones_col[:], 1.0)
    ones8 = singles.tile([8, 1], F32)
    nc.vector.memset(ones8[:], 1.0)

    csb = singles.tile([1, 8], F32)
    nc.sync.dma_start(csb[:], consts[:])
    bsel_sb = singles.tile([1, 8], F32)
    nc.sync.dma_start(bsel_sb[:], bsel[:])
    msel_sb = singles.tile([1, 8], F32)
    nc.sync.dma_start(msel_sb[:], msel[:])

    cb_ps = small.tile([128, 8], F32, tag="small")
    nc.tensor.matmul(cb_ps[:], ones_row[:], csb[:])
    cbc = singles.tile([128, 8], F32)
    nc.vector.tensor_copy(cbc[:], cb_ps[:])

    qwin_all = singles.tile([128, 8 * 384], BF16)
    nc.sync.dma_start(qwin_all[:], qwinT.rearrange("(c p) o -> p c o", p=128))

    def qwin(c, lo, hi):
        return qwin_all[:, c * 384 + lo:c * 384 + hi]

    qT_sb = singles.tile([128, TOK], BF16)
    kT_sb = singles.tile([128, TOK], BF16)
    vT_sb = singles.tile([128, TOK], BF16)

    qxT_r = qxT.rearrange("(c p) t -> p c t", p=128)

    # collective buffers: a2a1 blocks [64,512] (hl=0 y rows);
    # a2a2 blocks [66,512] (hl=1 y rows + stats hi/lo rows)
    a2a1_in = dram.tile([NCORES * 64, 512], BF16)
    a2a1_out = dram.tile([NCORES * 64, 512], BF16)
    a2a2_in = dram.tile([NCORES * 66, 512], BF16)
    a2a2_out = dram.tile([NCORES * 66, 512], BF16)
    a2a1_in_r = a2a1_in.rearrange("(bb h p) t -> p bb h t", p=64, h=2)
    a2a2_in_r = a2a2_in.rearrange("(j p) t -> p j t", p=66)

    yT_sb = singles.tile([128, TOK], BF16)
    stats = singles.tile([128, 9], F32)
    sq_tmp = singles.tile([128, 512], BF16)

    va = []
    for tb32 in range(32):
        t_ = singles.tile([128, 2 * (HD + 1)], BF16, tag=f"va{tb32}", name=f"va{tb32}")
        nc.vector.memset(t_[:, HD:HD + 1], 1.0)
        nc.vector.memset(t_[:, 2 * HD + 1:2 * HD + 2], 1.0)
        va.append(t_)

    def emit_qkv(b):
        for tb in (2 * b, 2 * b + 1):
            qx_tb = sb.tile([128, 8, 512], BF16, tag="qx", bufs=3, name=f"qx{tb}")
            if tb == 0:
                for c in range(8):
                    nc.sync.dma_start(qx_tb[:, c, :], qxT_r[:, c, 0:512])
            else:
                nc.sync.dma_start(qx_tb[:], qxT_r[:, :, tb * 512:(tb + 1) * 512])
            qk_ps = big.tile([128, 1024], F32, tag="big", name=f"qkps{tb}")
            v_ps = big.tile([128, 512], F32, tag="big", name=f"vps{tb}")
            for c in range(8):
                st, sp = (c == 0), (c == 7)
                nc.tensor.matmul(qk_ps[:, 0:512], qwin(c, 0, 128), qx_tb[:, c, :], start=st, stop=sp)
                nc.tensor.matmul(qk_ps[:, 512:1024], qwin(c, 128, 256), qx_tb[:, c, :], start=st, stop=sp)
                nc.tensor.matmul(v_ps[:], qwin(c, 256, 384), qx_tb[:, c, :], start=st, stop=sp)
            nc.vector.tensor_copy(qT_sb[:, tb * 512:(tb + 1) * 512], qk_ps[:, 0:512])
            nc.vector.tensor_copy(kT_sb[:, tb * 512:(tb + 1) * 512], qk_ps[:, 512:1024])
            nc.vector.tensor_copy(vT_sb[:, tb * 512:(tb + 1) * 512], v_ps[:])
        for tb32 in range(8 * b, 8 * b + 8):
            tr_ps = small.tile([128, 128], BF16, tag="small", name=f"vtr{tb32}")
            nc.tensor.transpose(tr_ps[:], vT_sb[:, tb32 * 128:(tb32 + 1) * 128], ident_bf[:])
            nc.vector.tensor_copy(va[tb32][:, 0:HD], tr_ps[:, 0:HD])
            nc.vector.tensor_copy(va[tb32][:, HD + 1:2 * HD + 1], tr_ps[:, HD:2 * HD])

    def emit_qk(hl, b, pair_idx):
        qrow = hl * HD
        tbase = b * T
        se_tiles = {}
        for ib in range(2):
            jb_max = 4 * ib + 3
            for jp in range(0, (jb_max + 1) // 2):
                jb0, jb1 = 2 * jp, 2 * jp + 1
                s_ps = big.tile([128, 1024], F32, tag="big", name=f"s_ps{pair_idx}_{ib}_{jp}")
                for col, jb in ((0, jb0), (512, jb1)):
                    nc.tensor.matmul(
                        s_ps[:, col:col + 512],
                        kT_sb[qrow:qrow + HD, tbase + jb * 128:tbase + (jb + 1) * 128],
                        qT_sb[qrow:qrow + HD, tbase + ib * 512:tbase + (ib + 1) * 512],
                    )
                se = sb.tile([128, 1024], BF16, tag="se", bufs=14, name=f"se{pair_idx}_{ib}_{jp}")
                nc.scalar.activation(se[:], s_ps[:], AF.Exp, scale=cbc[:, 0:1])
                for col, jb in ((0, jb0), (512, jb1)):
                    p = jb - 4 * ib
                    if 0 <= p <= 3:
                        dcol = col + 128 * p
                        nc.gpsimd.affine_select(
                            out=se[:, dcol:dcol + 128], in_=se[:, dcol:dcol + 128],
                            compare_op=ALU.is_ge, fill=0.0, base=0,
                            pattern=[[1, 128]], channel_multiplier=-1,
                        )
                se_tiles[(ib, jp)] = se
        return se_tiles

    def emit_pv(hl, b, pair_idx, se_tiles):
        tbase = b * T
        y_pair = sb.tile([128, 512], BF16, tag="ypair", bufs=4, name=f"ypair{pair_idx}")
        for ib128 in range(8):
            ib512 = ib128 // 4
            icol = 128 * (ib128 % 4)
            pv_ps = small.tile([128, HD + 1], F32, tag="small", name=f"pv{pair_idx}_{ib128}")
            for jb in range(ib128 + 1):
                se = se_tiles[(ib512, jb // 2)]
                lhs = se[:, 512 * (jb % 2) + icol: 512 * (jb % 2) + icol + 128]
                nc.tensor.matmul(
                    pv_ps[:], lhs, va[b * 8 + jb][:, (HD + 1) * hl:(HD + 1) * hl + HD + 1],
                    start=(jb == 0), stop=(jb == ib128),
                )
            rec = sb.tile([128, 1], F32, tag="rec", bufs=2, name=f"rec{pair_idx}_{ib128}")
            nc.vector.reciprocal(rec[:], pv_ps[:, HD:HD + 1])
            nc.vector.tensor_scalar_mul(y_pair[:, ib128 * HD:(ib128 + 1) * HD], pv_ps[:, 0:HD], rec[:])
        for ib128 in range(8):
            ytr_ps = small.tile([HD, 128], BF16, tag="small", name=f"ytr{pair_idx}_{ib128}")
            nc.tensor.transpose(ytr_ps[:], y_pair[:, ib128 * HD:(ib128 + 1) * HD], ident_bf[:])
            nc.vector.tensor_copy(
                yT_sb[hl * HD:(hl + 1) * HD, tbase + ib128 * 128:tbase + (ib128 + 1) * 128],
                ytr_ps[:],
            )
        if hl == 0:
            nc.sync.dma_start(a2a1_in_r[:, b, :, :], yT_sb[0:64, tbase:tbase + T])
        else:
            nc.sync.dma_start(a2a2_in_r[0:64, 2 * b:2 * b + 2, :], yT_sb[64:128, tbase:tbase + T])
        s1 = sb.tile([128, 1], F32, tag="st1", bufs=2, name=f"s1_{pair_idx}")
        nc.vector.reduce_sum(s1[:], y_pair[:], axis=AX.X)
        nc.vector.tensor_mul(sq_tmp[:], y_pair[:], y_pair[:])
        s2 = sb.tile([128, 1], F32, tag="st2", bufs=2, name=f"s2_{pair_idx}")
        nc.vector.reduce_sum(s2[:], sq_tmp[:], axis=AX.X)
        s3 = sb.tile([128, 1], F32, tag="st3", bufs=2, name=f"s3_{pair_idx}")
        nc.vector.reduce_max(s3[:], y_pair[:], axis=AX.X, apply_absolute_value=True)
        if hl == 0:
            nc.vector.tensor_copy(stats[:, b:b + 1], s1[:])
            nc.vector.tensor_copy(stats[:, 4 + b:5 + b], s2[:])
        else:
            nc.vector.tensor_add(stats[:, b:b + 1], stats[:, b:b + 1], s1[:])
            nc.vector.tensor_add(stats[:, 4 + b:5 + b], stats[:, 4 + b:5 + b], s2[:])
        if pair_idx == 0:
            nc.vector.tensor_copy(stats[:, 8:9], s3[:])
        else:
            nc.vector.tensor_max(stats[:, 8:9], stats[:, 8:9], s3[:])

    # ---- schedule: all hl=0 pairs first (their A2A overlaps hl=1 work) ----
    emit_qkv(0)
    se_prev = emit_qk(0, 0, 0)
    prev = (0, 0, 0, se_prev)
    pi = 1
    for b in range(1, 4):
        emit_qkv(b)
        se = emit_qk(0, b, pi)
        emit_pv(*prev)
        prev = (0, b, pi, se)
        pi += 1
    emit_pv(*prev)
    nc.gpsimd.collective_compute(
        "AllToAll", ALU.bypass, replica_groups=[list(range(NCORES))],
        ins=[a2a1_in.opt()], outs=[a2a1_out.opt()],
    )
    qwout_all = singles.tile([128, 8 * 1024], BF16)
    nc.sync.dma_start(qwout_all[:], qwoutT.rearrange("(c p) o -> p c o", p=128))
    qy = singles.tile([128, 8, 512], BF16)
    a2a1_o_r = a2a1_out.rearrange("(j p) t -> p j t", p=64)
    a2a2_o_r = a2a2_out.rearrange("(j p) t -> p j t", p=66)
    nc.sync.dma_start(qy[0:64, :, :], a2a1_o_r[:, :, :])

    def qwout(c, lo, hi):
        return qwout_all[:, c * 1024 + lo:c * 1024 + hi]

    prev = (1, 0, pi, emit_qk(1, 0, pi))
    pi += 1
    for b in range(1, 4):
        se = emit_qk(1, b, pi)
        emit_pv(*prev)
        prev = (1, b, pi, se)
        pi += 1
    emit_pv(*prev)

    # ---- stats rows + second A2A ----
    st_ps = small.tile([1, 9], F32, tag="small")
    nc.tensor.matmul(st_ps[:], ones_col[:], stats[:])
    trm_ps = small.tile([1, 128], F32, tag="small")
    nc.tensor.transpose(trm_ps[:], stats[:, 8:9], ident_f32[:])
    gmax_l = singles.tile([1, 1], F32)
    nc.vector.reduce_max(gmax_l[:], trm_ps[:], axis=AX.X)

    srow = singles.tile([1, 512], F32)
    nc.vector.memset(srow[:], 0.0)
    nc.vector.tensor_copy(srow[:, 0:8], st_ps[:, 0:8])
    nc.vector.tensor_scalar_mul(srow[:, 8:16], msel_sb[:], gmax_l[:])
    srep_ps = small.tile([8, 512], F32, tag="small")
    nc.tensor.matmul(srep_ps[:], ones_row[:, 0:8], srow[:])
    srep = singles.tile([8, 512], F32)
    nc.vector.tensor_copy(srep[:], srep_ps[:])
    hi8 = singles.tile([8, 512], BF16)
    nc.vector.tensor_copy(hi8[:], srep[:])
    hi8f = singles.tile([8, 512], F32)
    nc.vector.tensor_copy(hi8f[:], hi8[:])
    lo8 = singles.tile([8, 512], BF16)
    nc.vector.tensor_sub(lo8[:], srep[:], hi8f[:])
    nc.sync.dma_start(a2a2_in_r[64, :, :], hi8[:])
    nc.sync.dma_start(a2a2_in_r[65, :, :], lo8[:])
    nc.gpsimd.collective_compute(
        "AllToAll", ALU.bypass, replica_groups=[list(range(NCORES))],
        ins=[a2a2_in.opt()], outs=[a2a2_out.opt()],
    )

    # ---- global stats, quantize, output projection ----
    nc.sync.dma_start(qy[64:128, :, :], a2a2_o_r[0:64, :, :])
    sr_hi = singles.tile([8, 16], BF16)
    nc.sync.dma_start(sr_hi[:], a2a2_o_r[64, :, 0:16])
    sr_lo = singles.tile([8, 16], BF16)
    nc.sync.dma_start(sr_lo[:], a2a2_o_r[65, :, 0:16])
    stats_f = singles.tile([8, 16], F32)
    nc.vector.tensor_add(stats_f[:], sr_hi[:], sr_lo[:])

    glob_ps = small.tile([1, 16], F32, tag="small")
    nc.tensor.matmul(glob_ps[:], ones8[:], stats_f[:])
    sc = singles.tile([1, 24], F32)
    inv_tc = 1.0 / float(T * C)
    nc.vector.tensor_scalar_mul(sc[:, 0:8], glob_ps[:, 0:8], inv_tc)
    gmax = singles.tile([1, 1], F32)
    nc.vector.reduce_max(gmax[:], glob_ps[:, 8:16], axis=AX.X)
    nc.vector.tensor_mul(sc[:, 8:12], sc[:, 0:4], sc[:, 0:4])
    nc.vector.tensor_sub(sc[:, 8:12], sc[:, 4:8], sc[:, 8:12])
    nc.vector.tensor_scalar_add(sc[:, 8:12], sc[:, 8:12], 1e-5)
    sig = singles.tile([1, 4], F32)
    nc.scalar.activation(sig[:], sc[:, 8:12], AF.Sqrt)
    rsig = singles.tile([1, 4], F32)
    nc.vector.reciprocal(rsig[:], sig[:])
    nc.vector.tensor_scalar_mul(sc[:, 12:16], rsig[:], csb[:, 1:2])
    nc.vector.tensor_mul(sc[:, 16:20], sc[:, 0:4], sc[:, 12:16])
    tsel = singles.tile([1, 8], F32)
    nc.vector.tensor_mul(tsel[:], sc[:, 12:20], bsel_sb[:])
    row4 = singles.tile([1, 4], F32)
    nc.vector.reduce_sum(row4[:, 0:2], tsel.rearrange("p (g f) -> p g f", g=2), axis=AX.X)
    nc.vector.tensor_scalar_mul(row4[:, 3:4], gmax[:], csb[:, 2:3])
    nc.vector.tensor_scalar_mul(row4[:, 2:3], row4[:, 3:4], -1.0)
    qsc_ps = small.tile([128, 4], F32, tag="small")
    nc.tensor.matmul(qsc_ps[:], ones_row[:], row4[:])
    qsc = singles.tile([128, 4], F32)
    nc.vector.tensor_copy(qsc[:], qsc_ps[:])

    qy_flat = qy.rearrange("p j t -> p (j t)")
    nc.vector.tensor_scalar(
        out=qy_flat, in0=qy_flat, scalar1=qsc[:, 0:1], scalar2=qsc[:, 1:2],
        op0=ALU.mult, op1=ALU.subtract,
    )
    nc.vector.tensor_scalar(
        out=qy_flat, in0=qy_flat, scalar1=qsc[:, 2:3], scalar2=qsc[:, 3:4],
        op0=ALU.max, op1=ALU.min,
    )

    for tch in range(4):
        osb = sb.tile([128, 1024], F32, tag="ob", bufs=2, name=f"osb{tch}")
        for oh in range(2):
            o_ps = big.tile([128, 512], F32, tag="big", name=f"ops{tch}_{oh}")
            for cj in range(8):
                nc.tensor.matmul(
                    o_ps[:], qy[:, cj, tch * 128:(tch + 1) * 128],
                    qwout(cj, oh * 512, (oh + 1) * 512),
                    start=(cj == 0), stop=(cj == 7),
                )
            nc.vector.tensor_copy(osb[:, oh * 512:(oh + 1) * 512], o_ps[:])
        nc.sync.dma_start(out[tch * 128:(tch + 1) * 128, :], osb[:])


@functools.lru_cache(maxsize=1)
def build():
    nc = bacc.Bacc(None)
    with tile.TileContext(nc) as tc:
        with ExitStack() as ctx:
            _emit(nc, tc, ctx)
    nc.finalize()
    return nc


def _host_prep(x, w_in, w_out):
    x = np.asarray(x, np.float32)
    w_in = np.asarray(w_in, np.float32)
    w_out = np.asarray(w_out, np.float32)

    a1 = w_in.mean()
    qw1 = np.sign(w_in - a1).astype(np.float32)
    b1 = np.abs(w_in).mean()
    a2 = w_out.mean()
    qw2 = np.sign(w_out - a2).astype(np.float32)
    b2 = np.abs(w_out).mean()

    mu = x.mean(axis=(1, 2), keepdims=True)
    var = x.var(axis=(1, 2), keepdims=True)
    g1 = np.abs(x).max()
    xn = (x - mu) / np.sqrt(var + 1e-5)
    qx = np.clip(xn * (QB / g1), -QB + EPS, QB - EPS)
    scale1 = b1 * g1 / QB

    bf = ml_dtypes.bfloat16
    qxT = np.ascontiguousarray(qx.reshape(TOK, C).T).astype(bf)
    qwoutT = np.ascontiguousarray(qw2.T).astype(bf)
    att_scale = scale1 * scale1 / math.sqrt(HD)
    cbound = (QB - EPS) / QB * b2 * scale1
    consts = np.array([[att_scale, b2, cbound, 0, 0, 0, 0, 0]], np.float32)

    in_maps = []
    for core in range(NCORES):
        r0 = core * 128
        qwin = np.concatenate(
            [qw1[r0:r0 + 128], qw1[C + r0:C + r0 + 128], qw1[2 * C + r0:2 * C + r0 + 128]], axis=0
        )
        qwinT = np.ascontiguousarray(qwin.T).astype(bf)
        bsel_ = np.zeros((1, 8), np.float32)
        bsel_[0, core // 2] = 1.0
        bsel_[0, 4 + core // 2] = 1.0
        msel_ = np.zeros((1, 8), np.float32)
        msel_[0, core] = 1.0
        in_maps.append({
            "qxT": qxT, "qwinT": qwinT, "qwoutT": qwoutT,
            "consts": consts, "bsel": bsel_, "msel": msel_,
        })
    return in_maps


def kernel(x, w_in, w_out):
    in_maps = _host_prep(x, w_in, w_out)
    nc = build()
    res = run_bass_kernel_spmd(nc, in_maps, core_ids=list(range(NCORES)))
    out = np.concatenate([np.asarray(res.results[i]["out"]) for i in range(NCORES)], axis=0)
    return out.reshape(B, T, C).astype(np.float32)
